# revision 44
# baseline (speedup 1.0000x reference)
"""Trainium2 Bass kernel for nn_CC2TBAELoss (data-parallel loss over n=20000).

Strategy: pure data parallelism over 8 NeuronCores (2500 samples each, padded
to 2560 = 20 tiles of 128 samples). Each core streams its shard once from HBM
and produces per-partition partial sums for the 5 loss terms; the host does the
final (tiny) reduction and weighting.

Perf structure (v6). The kernel is HBM-bound (~6.99 MB per 128-sample tile,
~17.5us/tile at ~400 GB/s), so the whole design serves one goal: the 16 SDMA
engines must never idle. Three independent issue rings, each with enough
buffer depth that its issues run 1-3 tiles ahead of compute:
  - sync (SP HWDGE) ring: cov, obs, H per tile (4-8 KB descriptors at line
    rate) + dpi batched 4 tiles per DMA (issued 1-2 tiles early) + one-shot
    drift load in the prologue.
  - scalar (ACT HWDGE) ring: encoder_hessian in quarter-tile (1 MB) chunks,
    one 6-deep rotation; quarters of tile t+1 issue during t, and squares
    run split (trailing quarters of t-1 early in t, leading quarters of t
    late in t) -> the enc stream drains IN the loop (no multi-tile ACT
    tail) and every issue's buffer was freed by an already-emitted square.
  - gpsimd (SWDGE) ring: model_projection two tiles ahead (its only
    consumer is the gpsimd tangent sub -> private self-paced loop) +
    one-shot flat x_hat/x loads in the prologue (2.5 KB contiguous
    descriptors; mse is layout-agnostic).
Engine roles (the key lesson of this kernel: the latency-critical chain
must live on ONE engine so it can never stall on a cross-engine hop, and
every cross-engine edge that remains gets >= 1 tile of slack):
  - DVE owns the whole curvature chain: transposes (t), bbt transpose
    (t+1), and the qv/npv/accumulate block (t+2) as one zero-wait unit.
    The P*t product writes to scratch, NOT in-place on obs, so the chain
    has no WAR edge on the tangent sub.
  - GPSIMD: H*bbt multiply at t+1 (a full tile ahead of its DVE consumer)
    + the tangent sub; SWDGE issues go last in its program order.
  - ACT: all squares (enc, dpi batch, tangent, mse) via
    activation(Square, accum_out=...) + the MT PSUM->SBUF copy.
  - PE: the two per-sample matmul steps (t and t+1); step2 writes PSUM
    without a memset (the garbage rows land in bbt columns that are
    sliced away).

Per 128-sample tile (sample index s in [0,128); R = s//32, u = s%32):
  - mse/contractive/hessian/tangent terms: fused square(+diff)+accumulate,
    single pass per element, layout-agnostic.
  - curvature term: per-sample small matmuls on the TensorEngine, packed via
    tile_position 32x32 sub-arrays. All tiles are written by exactly one DMA
    or one compute op (keeps the race detector provably clean):
      dpiT  = DVE 32x32 block-transpose of the natural dpi tile
              -> dpiT[32R + a, 32dd + u] = dpi[s, dd, a]
      covT  = DVE 32x32 block-transpose of the natural cov tile
              -> covT[32R + b, 32a + u] = cov[s, a, b]
      step1 M'_s = cov_s @ dpi_s^T     (covT slice stationary, dpiT moving)
              -> psum_M[32R + a, 8u + dd] = M'_s[a, dd]
      step2 bbt_s = dpi_s @ M'_s       (dpiT slice stationary, M' moving),
            written to PSUM with strided columns (diagonal tile_position --
            off-diagonal col groups crash this HW/toolchain):
              psum_b[32R + j, 32k + u] = bbt_s[j, k]
    so ONE DVE 32x32 block-transpose lands bbt in NATURAL
    samples-on-partitions layout: bbt_sp[s, 32k + j] = bbt_s[j, k].
    All other per-sample tensors load naturally; qv / tangent_vector / npv
    are plain DVE broadcast-multiply + segmented reductions per partition.
  Software pipeline: step1(t) at iteration t, step2 (b_stage) at t+1,
  qv/npv chain (c_stage) at t+2.
"""

import os
import sys

import numpy as np

for _p in ("/opt/trn_rl_repo", "/root/.axon_site/_ro/trn_rl_repo"):
    if os.path.isdir(_p) and _p not in sys.path:
        sys.path.insert(0, _p)

import concourse.bacc as bacc
import concourse.bass as bass
import concourse.tile as tile
from concourse import mybir
from concourse.bass_utils import run_bass_kernel_spmd

F32 = mybir.dt.float32
N_TOTAL = 20000
D = 32
DD = 8
N_CORES = 8
P = 128
PER_CORE_PAD = 2560  # 2500 padded up to a multiple of 128

_CACHE = {}


def _build(n_per_core: int) -> bass.Bass:
    assert n_per_core % (4 * P) == 0
    nt = n_per_core // P
    nb = nt // 4  # 4-tile batches for dpi / drift
    nc = bacc.Bacc("TRN2")

    shapes = {
        "x_hat": [n_per_core, D],
        "dpi": [n_per_core, DD, D],
        "model_projection": [n_per_core, D, D],
        "decoder_hessian": [n_per_core, D, DD, DD],
        "encoder_hessian": [n_per_core, DD, D, D],
        "x": [n_per_core, D],
        "ambient_drift": [n_per_core, D],
        "ambient_cov": [n_per_core, D, D],
        "observed_projection": [n_per_core, D, D],
    }
    ins = {
        k: nc.dram_tensor(k, shp, F32, kind="ExternalInput").ap()
        for k, shp in shapes.items()
    }
    out = nc.dram_tensor("out", [P, 8], F32, kind="ExternalOutput").ap()

    AX = mybir.AxisListType
    OP = mybir.AluOpType
    ACTF = mybir.ActivationFunctionType

    FLAT = n_per_core * D // P  # flat cols/partition for x_hat / x
    ECH = 2048                  # enc quarter-tile cols (8 KB/partition, 1 MB)

    with tile.TileContext(nc) as tc:
        with (
            tc.tile_pool(name="io", bufs=3) as io,
            tc.tile_pool(name="deriv", bufs=2) as dv,
            tc.tile_pool(name="accp", bufs=1) as accp,
            tc.tile_pool(name="psum", bufs=2, space="PSUM") as psp,
        ):
            zbias = accp.tile([P, 1], F32, tag="zbias")
            nc.vector.memset(zbias, 0.0)
            acc_mse = accp.tile([P, 1], F32, tag="acc_mse")
            acc_dpi = accp.tile([P, nb], F32, tag="acc_dpi")
            acc_enc = accp.tile([P, 4 * nt], F32, tag="acc_enc")
            acc_tang = accp.tile([P, nt], F32, tag="acc_tang")
            acc_curv = accp.tile([P, nt], F32, tag="acc_curv")

            # ---------------- prologue DMAs ----------------
            # gpsimd (SWDGE) ring: mdl tile 0, then one-shot flat x_hat/x.
            mdl_bufs = {}

            def mdl_issue(tau):
                m_t = io.tile([P, D * D], F32, tag="mdl", bufs=4)
                nc.gpsimd.dma_start(
                    out=m_t,
                    in_=ins["model_projection"][tau * P : (tau + 1) * P]
                    .rearrange("s i j -> s (i j)"),
                )
                mdl_bufs[tau] = m_t

            mdl_issue(0)
            mdl_issue(1)
            xh_f = accp.tile([P, FLAT], F32, tag="xh_f")
            nc.gpsimd.dma_start(
                out=xh_f, in_=ins["x_hat"].rearrange("(p c) d -> p (c d)", p=P)
            )
            x_f = accp.tile([P, FLAT], F32, tag="x_f")
            nc.gpsimd.dma_start(
                out=x_f, in_=ins["x"].rearrange("(p c) d -> p (c d)", p=P)
            )

            # 4-tile-batched dpi loads on the sync ring; drift is tiny
            # (2.5 KB/partition total) so it loads once in the prologue,
            # in per-sample layout.
            dpi_bufs = {}

            def batch_issue(b):
                bsl = slice(b * 4 * P, (b + 1) * 4 * P)
                d_t = io.tile([P, 4 * DD * D], F32, tag="dpib", bufs=2)
                nc.sync.dma_start(
                    out=d_t.rearrange("p (t f) -> p t f", t=4),
                    in_=ins["dpi"][bsl].rearrange(
                        "(t p) dd a -> p t (dd a)", t=4
                    ),
                )
                dpi_bufs[b] = d_t

            dr_all = accp.tile([P, nt * D], F32, tag="dr_all")
            nc.sync.dma_start(
                out=dr_all.rearrange("p (t d) -> p t d", t=nt),
                in_=ins["ambient_drift"].rearrange("(t p) d -> p t d", t=nt),
            )
            batch_issue(0)

            # enc quarter-tile (1 MB) issues on the scalar (ACT HWDGE) ring.
            # Single 6-deep rotation; squares run split (quarters 2-3 of tile
            # t-1 early in iter t, quarters 0-1 of tile t late in iter t) so
            # each issue's buffer was freed by a square already emitted.
            enc_bufs = {}

            def enc_issue_q(tau, qs):
                esl = slice(tau * P, (tau + 1) * P)
                esrc = ins["encoder_hessian"][esl].rearrange(
                    "s a b c -> s (a b c)"
                )
                for q in qs:
                    e_t = io.tile([P, ECH], F32, tag="enc", bufs=5)
                    nc.scalar.dma_start(
                        out=e_t, in_=esrc[:, q * ECH : (q + 1) * ECH]
                    )
                    enc_bufs[(tau, q)] = e_t

            enc_issue_q(0, (0, 1, 2, 3))

            def enc_square(tau, q):
                e_t = enc_bufs.pop((tau, q))
                nc.scalar.activation(
                    out=e_t,
                    in_=e_t,
                    func=ACTF.Square,
                    bias=zbias,
                    accum_out=acc_enc[:, 4 * tau + q : 4 * tau + q + 1],
                )

            # ---------------- pipeline stages ----------------
            def b_stage(pv):
                dpiT_v = pv["dpiT_v"]
                MT_t = pv["MT_t"]
                # step2: bbt_s = dpi_s @ M'_s -> psum_b[32R + j, 32k + u]
                # (no memset: rows 32R+8..32R+31 stay garbage, but the bbt
                # transpose lands them in columns j>=8 which bbt_v slices
                # away -- they are never read.)
                psum_b = psp.tile([P, 256], F32, tag="pb")
                pbv = psum_b.rearrange("p (k w) -> p k w", k=8)
                for u in range(32):
                    for R in range(4):
                        nc.tensor.matmul(
                            out=pbv[32 * R : 32 * R + 8, :, u],
                            lhsT=dpiT_v[32 * R : 32 * R + 32, :, u],
                            rhs=MT_t[32 * R : 32 * R + 32, 8 * u : 8 * u + 8],
                            start=True,
                            stop=True,
                            tile_position=(32 * R, 32 * R),
                        )
                # bbt_sp[s, 32k + j] = bbt_s[j, k]  (s natural = 32R + u)
                # Split transpose + split H*bbt multiply.  The multiply runs
                # on GpSimd a full tile ahead of its consumer (c_stage at
                # t+1), so the cross-engine hop has a tile of slack and can
                # never pace the pipeline.
                bbt_sp = dv.tile([P, 256], F32, tag="bbt_sp", bufs=3)
                H_t = pv["H_t"]
                H4 = H_t.rearrange("p (i k j) -> p i k j", i=32, k=8)
                bbt_v = bbt_sp.rearrange("p (k j) -> p k j", k=8)[:, :, 0:8]
                bbt_b = bbt_v[:, None, :, :].broadcast_to((P, 32, 8, 8))
                for hh in range(2):
                    csl = slice(128 * hh, 128 * hh + 128)
                    nc.vector.transpose(
                        out=bbt_sp[:, csl], in_=psum_b[:, csl]
                    )
                    ksl = slice(4 * hh, 4 * hh + 4)
                    nc.gpsimd.tensor_mul(
                        H4[:, :, ksl, :], H4[:, :, ksl, :], bbt_b[:, :, ksl, :]
                    )

            def c_stage(pv):
                # The qv -> npv chain for tile t-2 as ONE uninterrupted DVE
                # block: every input (H*bbt from GpSimd at t-1, obs, dr) is
                # >=1 tile old, so this never stalls mid-block.
                H_t = pv["H_t"]
                obs_t = pv["obs_t"]
                dr_t = pv["dr_t"]
                tp = pv["tcol"]
                qv_t = dv.tile([P, D], F32, tag="qv")
                nc.vector.tensor_reduce(
                    out=qv_t,
                    in_=H_t.rearrange("p (i q) -> p i q", i=32),
                    axis=AX.X,
                    op=OP.add,
                )
                tt = dv.tile([P, D], F32, tag="tt")
                nc.vector.scalar_tensor_tensor(
                    out=tt, in0=qv_t, scalar=-0.5, in1=dr_t,
                    op0=OP.mult, op1=OP.add,
                )
                # P*t product goes to a scratch tile (NOT in-place on obs):
                # in-place would add a WAR edge on the gpsimd tangent sub,
                # chaining this DVE block to the mdl DMA.
                obs3 = obs_t.rearrange("p (r i) -> p r i", r=32)
                t_b = tt[:, None, :].broadcast_to((P, 32, 32))
                scr3 = dv.tile([P, D * D], F32, tag="scr3", bufs=1)
                sc3 = scr3.rearrange("p (r i) -> p r i", r=32)
                nc.vector.tensor_mul(sc3, obs3, t_b)
                Pt_t = dv.tile([P, D], F32, tag="Pt")
                nc.vector.tensor_reduce(out=Pt_t, in_=sc3, axis=AX.X, op=OP.add)
                npv_t = dv.tile([P, D], F32, tag="npv")
                nc.vector.scalar_tensor_tensor(
                    out=npv_t, in0=Pt_t, scalar=-1.0, in1=tt,
                    op0=OP.mult, op1=OP.add,
                )
                scr2 = dv.tile([P, D], F32, tag="scr2")
                nc.vector.tensor_mul(scr2, npv_t, npv_t)
                nc.vector.tensor_reduce(
                    out=acc_curv[:, tp : tp + 1], in_=scr2, axis=AX.X, op=OP.add
                )

            def tang_stage(pv):
                # square+accumulate of LAST tile's (mdl-obs) on ACT (keeps
                # DVE free for the curvature chain).
                mdl_t = pv["mdl_t"]
                tp = pv["tcol"]
                nc.scalar.activation(
                    out=mdl_t,
                    in_=mdl_t,
                    func=ACTF.Square,
                    bias=zbias,
                    accum_out=acc_tang[:, tp : tp + 1],
                )

            def mt_copy(pv):
                # PSUM->SBUF move of LAST tile's step1 output, done at the
                # START of the next iteration: step1(t-1) finished during the
                # previous window, so these never stall ACT on the PE burst.
                psum_M = pv.pop("psum_M")
                MT_t = dv.tile([P, 256], F32, tag="MT", bufs=3)
                for q in range(4):
                    qsl = slice(64 * q, 64 * q + 64)
                    nc.scalar.copy(out=MT_t[:, qsl], in_=psum_M[:, qsl])
                pv["MT_t"] = MT_t

            prev = None
            prev2 = None
            prevtang = None
            for t in range(nt):
                sl = slice(t * P, (t + 1) * P)
                b, t4 = divmod(t, 4)

                # ---- sync-ring DMAs (PE/DVE-feeding tiles first).
                if t4 == 2 and b + 1 < nb:
                    batch_issue(b + 1)
                cov_t = io.tile([P, D * D], F32, tag="cov", bufs=6)
                nc.sync.dma_start(
                    out=cov_t,
                    in_=ins["ambient_cov"][sl].rearrange("s a b -> s (a b)"),
                )
                obs_t = io.tile([P, D * D], F32, tag="obs", bufs=6)
                nc.sync.dma_start(
                    out=obs_t,
                    in_=ins["observed_projection"][sl].rearrange("s i j -> s (i j)"),
                )
                H_t = io.tile([P, D * DD * DD], F32, tag="H", bufs=7)
                nc.sync.dma_start(
                    out=H_t,
                    in_=ins["decoder_hessian"][sl].rearrange("s i k j -> s (i k j)"),
                )
                # ACT: trailing enc quarters of tile t-1, dpi square, MT copy,
                # leading enc quarters of tile t, then the t+1 enc issues.
                dpi_t = dpi_bufs[b][:, t4 * DD * D : (t4 + 1) * DD * D]
                if t4 == 0:
                    dpisq = accp.tile([P, 4 * DD * D], F32, tag="dpisq")
                    nc.scalar.activation(
                        out=dpisq,
                        in_=dpi_bufs[b],
                        func=ACTF.Square,
                        bias=zbias,
                        accum_out=acc_dpi[:, b : b + 1],
                    )
                # 5-buf enc rotation cadence: square (t-1,3) early, (t,0..2)
                # late; quarter 3 of t+1 issues only after (t,2)'s square
                # frees its slot (same-engine order keeps this lock-free).
                if t >= 1:
                    enc_square(t - 1, 3)
                if prev is not None:
                    mt_copy(prev)
                enc_square(t, 0)
                enc_square(t, 1)
                if t + 1 < nt:
                    enc_issue_q(t + 1, (0, 1, 2))
                    enc_square(t, 2)
                    enc_issue_q(t + 1, (3,))
                else:
                    # last tile: its trailing quarters landed long ago.
                    enc_square(t, 2)
                    enc_square(t, 3)
                if prevtang is not None:
                    tang_stage(prevtang)
                    prevtang = None

                # qv/npv chain for tile t-2 (single DVE block) runs FIRST
                # in DVE program order: its inputs are all >=1 tile old, so
                # DVE starts the iteration with zero waits.
                if prev2 is not None:
                    c_stage(prev2)
                    prev2 = None

                dpiT_t = dv.tile([P, DD * D], F32, tag="dpiT", bufs=3)
                nc.vector.transpose(out=dpiT_t, in_=dpi_t)
                # dpiT_t[32R + a, 32dd + u] = dpi[32R + u, dd, a]
                covT_t = dv.tile([P, D * D], F32, tag="covT", bufs=2)
                nc.vector.transpose(out=covT_t, in_=cov_t)
                # covT_t[32R + b, 32a + u] = cov[32R + u, a, b]
                dpiT_v = dpiT_t.rearrange("p (dd u) -> p dd u", dd=8)
                covT_v = covT_t.rearrange("p (a u) -> p a u", a=32)

                # mse once, after the prologue flat loads have landed:
                # DVE sub, then ACT square+accumulate.
                if t == 1:
                    diff = accp.tile([P, FLAT], F32, tag="diff")
                    nc.vector.tensor_sub(diff, xh_f, x_f)
                    nc.scalar.activation(
                        out=diff,
                        in_=diff,
                        func=ACTF.Square,
                        bias=zbias,
                        accum_out=acc_mse,
                    )

                # B-stage for tile t-1: PE2 + bbt PSUM->SBUF transpose.
                if prev is not None:
                    b_stage(prev)
                    prev2 = prev
                    prev = None
                    if t == nt - 1:
                        # pull tile nt-2's qv chain into the last iteration
                        # (overlaps the final sync-ring loads).
                        c_stage(prev2)
                        prev2 = None

                # step1: M'_s = cov_s @ dpi_s^T  -> psum_M[32R + a, 8u + dd]
                psum_M = psp.tile([P, 256], F32, tag="pm")
                for u in range(32):
                    for R in range(4):
                        nc.tensor.matmul(
                            out=psum_M[32 * R : 32 * R + 32, 8 * u : 8 * u + 8],
                            lhsT=covT_v[32 * R : 32 * R + 32, :, u],
                            rhs=dpiT_v[32 * R : 32 * R + 32, :, u],
                            start=True,
                            stop=True,
                            tile_position=(32 * R, 32 * R),
                        )
                prev = dict(
                    dpiT_v=dpiT_v, psum_M=psum_M, H_t=H_t, obs_t=obs_t,
                    dr_t=dr_all[:, t * D : (t + 1) * D], tcol=t,
                )

                # tangent sub late in gpsimd program order (it waits on the
                # mdl/obs DMAs); the next SWDGE mdl issue goes AFTER it so a
                # stalled issue can never head-of-line block the H*bbt muls.
                mdl_t = mdl_bufs.pop(t)
                nc.gpsimd.tensor_sub(mdl_t, mdl_t, obs_t)
                prevtang = dict(mdl_t=mdl_t, tcol=t)
                # mdl two tiles ahead (the sub must never wait on a
                # just-issued SWDGE transfer).
                if t + 2 < nt:
                    mdl_issue(t + 2)

            # Epilogue: MT copy first (its PE input is long done), then the
            # last tile's step2 + qv chain; the last tangent square runs on
            # ACT concurrently.
            if prev is not None:
                mt_copy(prev)
            if prevtang is not None:
                tang_stage(prevtang)
                prevtang = None
            if prev2 is not None:
                c_stage(prev2)
                prev2 = None
            if prev is not None:
                b_stage(prev)
                c_stage(prev)
                prev = None

            # ------------- final packing -------------
            outsb = accp.tile([P, 8], F32, tag="outsb")
            nc.vector.memset(outsb, 0.0)
            for j, acc in enumerate([acc_mse, acc_dpi, acc_enc, acc_tang, acc_curv]):
                nc.vector.tensor_reduce(
                    out=outsb[:, j : j + 1], in_=acc, axis=AX.X, op=OP.add
                )
            nc.sync.dma_start(out=out, in_=outsb)

    nc.finalize()
    return nc


def _get_nc(n_per_core: int) -> bass.Bass:
    if n_per_core not in _CACHE:
        _CACHE[n_per_core] = _build(n_per_core)
    return _CACHE[n_per_core]


def _make_in_maps(inputs: dict, per: int, nper: int) -> list[dict]:
    in_maps = []
    for ci in range(N_CORES):
        m = {}
        for k, arr in inputs.items():
            a = np.asarray(arr)[ci * per : (ci + 1) * per].astype(
                np.float32, copy=False
            )
            if nper > per:
                pad = np.zeros((nper - per,) + a.shape[1:], np.float32)
                a = np.concatenate([a, pad], axis=0)
            m[k] = np.ascontiguousarray(a)
        in_maps.append(m)
    return in_maps


def _combine(results, n_total: int) -> np.ndarray:
    parts = np.stack([r["out"] for r in results]).astype(np.float64)
    s = parts.sum(axis=(0, 1))
    loss = s[0] / (n_total * D) + (s[1] + s[2] + s[3]) / n_total + s[4]
    return np.array(loss, dtype=np.float32)


def run(inputs: dict, trace: bool = False):
    """Returns (loss, exec_time_ns or None). Used by kernel() and test.py."""
    n_total = np.asarray(inputs["x_hat"]).shape[0]
    per = n_total // N_CORES
    nper = ((per + P - 1) // P) * P
    nc = _get_nc(nper)
    in_maps = _make_in_maps(inputs, per, nper)
    res = run_bass_kernel_spmd(
        nc, in_maps, core_ids=list(range(N_CORES)), trace=trace
    )
    return _combine(res.results, n_total), res.exec_time_ns


def kernel(**inputs) -> np.ndarray:
    loss, _ = run(inputs)
    return loss
